# revision 11
# baseline (speedup 1.0000x reference)
"""Contrastive-loss kernel for trn2 (8 NeuronCores, SPMD), v4.

The reference loss reduces to a Gram matrix G = F.T @ F over the
flattened input F [N=524288, T=64], followed by a tiny [64,64] masked
margin reduction (host).  Changes vs the 69us baseline:

1. Host-side cast fp32 -> fp8 e4m3 (ml_dtypes.float8_e4m3, the TRN
   fp8e4 format, max +-240).  Device HBM traffic drops 4x to 4.19
   MiB/core (fully drains in ~11.5us at ~368 GB/s/side, all 16 SDMA
   engines ~97% busy); end-to-end loss rel-err ~7e-4 (tolerance 2e-2).
   The PE is the bottleneck.
2. PE warm-up: the HAM clock gate keeps the PE at 1.2 GHz until it has
   been busy for a full ~3.4us activity window (trace: 55 cold matmuls
   = 5.9us wasted).  A handful of junk matmuls on an uninitialized
   SBUF scratch (into a scratch PSUM bank nobody reads) fill the
   NRT-preamble -> first-tile-landed dead window so the HAM flip
   happens during warm-up, not mid-stream.
3. Packed matmuls: lhsT = rhs = [A|B] ([128,128] fp8 -> FWL),
   accumulating [[A'A,A'B],[B'A,B'B]] into one [128,128] PSUM tile;
   diagonal blocks summed by DVE (copy+add) at the end.  256 matmuls
   at ~50ns warm cadence.
4. No nc.Block(): instructions are emitted straight into the entry
   block, which removes the per-engine branch (and its ~0.9us ifetch
   stall on gpsimd right before the first DMA) and the walrus
   end-of-block barrier.
5. Minimal tail: the NRT-injected postamble zeroes ALL 256 semaphores
   (5 engines x 51 sems, trace-verified), so the kernel does no sem
   clearing of its own.  The output store's mandatory semaphore update
   (walrus crashes on a DMACopy with an empty update list) lands on
   forged low sem S[48], which nothing waits on and the postamble
   zeroes anyway -- no engine ever waits for the store's HBM write
   receipt.  gpsimd resets the SWDGE queue state after the last input
   DMA completes, off the critical path.

The 8 partial [64,64] Grams are summed on the host, where the masked
margin reduction (negligible work) also runs.
"""

import contextlib

import numpy as np
import ml_dtypes

import bass_rust
import concourse.bacc as bacc
import concourse.mybir as mybir
from concourse.bass_utils import run_bass_kernel_spmd

# See item 5 above: receipt sink for the output store.  S[48] is only
# ever touched by the NRT postamble reset chains (trace-verified).
_STORE_SEM = bass_rust.SemaphoreHandle("nrt_scratch", 48)

MARGIN = 60000.0
S = 64                           # time steps (Gram dim)
N_TOTAL = 2 * 8 * 32 * 32 * 32   # 524288 flattened rows
N_CORES = 8
N_SHARD = N_TOTAL // N_CORES     # 65536 rows per core
P = 128                          # SBUF partitions
# Tile sizes in rows (multiples of 256 so each tile is a whole number
# of packed [128,128] matmuls).  Equal mid-size tiles: a tile's landing
# time has a ~1.2us fixed floor (per-descriptor cost, 8 descs/engine),
# so tiny lead-in tiles land no sooner than an 8192-row tile but leave
# the PE starved at every boundary (v4 trace: 1.8/1.6/1.0us stalls that
# also re-cooled the HAM clock gate).  The junk warm-up (below) covers
# the lead-in instead, and 8192-row tiles keep the per-tile DMA time
# (~1.4us) below the warm PE time per tile (~1.66us) so the stream
# never stalls after tile 0.  Last tiles slightly bigger for margin.
TILE_ROWS = [4096, 8192, 8192, 8192, 8192, 8192, 8192, 8192, 4096]
# The first N_SYNC_TILES are DMA'd from the SP sequencer (HWDGE): SP's
# NRT preamble ends ~1us before gpsimd finishes emitting its first
# SWDGE descriptors, so tile 0 lands earlier and the real matmuls
# start sooner.  The rest stream via gpsimd SWDGE as before.
N_SYNC_TILES = 2
assert sum(TILE_ROWS) == N_SHARD and all(r % 256 == 0 for r in TILE_ROWS)
TILE_FREE = [(r // P) * S for r in TILE_ROWS]   # fp8 elems per partition
TILE_OFF = [sum(TILE_FREE[:i]) for i in range(len(TILE_ROWS))]
XBUF_FREE = sum(TILE_FREE)                      # 32768 B/partition (fp8)
N_TILES = len(TILE_ROWS)
# Junk warm-up matmuls: bridge PE-preamble-end -> tile0-landed (~2.5us)
# with N=512 matmuls (427ns each at the cold 1.2 GHz clock); the HAM
# window flips to 2.4 GHz shortly into the real stream.
N_JUNK = 6

_CACHE = {}
LAST_RESULTS = None              # BassKernelResults of the most recent run


def _build_nc():
    nc = bacc.Bacc("TRN2", target_bir_lowering=False, debug=False,
                   num_devices=N_CORES)
    # Drop the const-AP memsets and the all-engine barrier that
    # Bass.__init__ appends to the entry block (~0.5us before the first
    # kernel instruction can issue).  Nothing in this kernel uses the
    # const APs, and all cross-engine ordering is explicit via sems.
    entry = nc.main_func.blocks[0]
    first_memset = next(i for i, inst in enumerate(entry.instructions)
                        if isinstance(inst, mybir.InstMemset))
    del entry.instructions[first_memset:]

    x = nc.dram_tensor("x", [N_SHARD, S], mybir.dt.float8e4,
                       kind="ExternalInput")
    g = nc.dram_tensor("g", [S, S], mybir.dt.float32, kind="ExternalOutput")

    def tile_src(i):
        a = sum(TILE_ROWS[:i])
        return x[a:a + TILE_ROWS[i]].rearrange(
            "(p r) c -> p (r c)", p=P, r=TILE_ROWS[i] // P)

    with (
        nc.sbuf_tensor("xbuf", [P, XBUF_FREE], mybir.dt.float8e4) as xbuf,
        nc.sbuf_tensor("junk", [P, 512], mybir.dt.float8e4) as junk,
        nc.psum_tensor("acc", [2 * S, 2 * S], mybir.dt.float32) as acc,
        nc.psum_tensor("scr", [P, 512], mybir.dt.float32) as scr,
        nc.sbuf_tensor("obuf", [S, S], mybir.dt.float32) as obuf,
        nc.semaphore("pe_sem") as pe_sem,
        nc.semaphore("out_sem") as out_sem,
        contextlib.ExitStack() as stack,
    ):
        dma_sems = [stack.enter_context(nc.semaphore(f"dma_sem{k}"))
                    for k in range(N_TILES)]
        dma_lo = min(s.num for s in dma_sems)
        dma_hi = max(s.num for s in dma_sems)
        assert dma_hi - dma_lo == N_TILES - 1

        # --- SP: the first tiles via HWDGE (earliest possible start).
        for i in range(N_SYNC_TILES):
            nc.sync.dma_start(
                xbuf[:, TILE_OFF[i]:TILE_OFF[i] + TILE_FREE[i]],
                tile_src(i),
            ).then_inc(dma_sems[i], 16)
        # --- gpsimd: the bulk of the input via SWDGE, then reset the
        # SWDGE queue state.
        for i in range(N_SYNC_TILES, N_TILES):
            nc.gpsimd.dma_start(
                xbuf[:, TILE_OFF[i]:TILE_OFF[i] + TILE_FREE[i]],
                tile_src(i),
            ).then_inc(dma_sems[i], 16)
        # dma_sems[-1] == 16 implies every engine drained its FIFO
        # through the last tile, i.e. all SWDGE input DMAs completed (it
        # does NOT touch sem values, so the PE's pending per-tile waits
        # are unaffected).
        nc.gpsimd.wait_ge(dma_sems[-1], 16)
        nc.gpsimd.dma_reset(range(dma_lo, dma_hi + 1))

        # --- PE: junk warm-up (uninitialized operands, scratch PSUM,
        # nobody reads the result -- only the HAM activity matters).
        for j in range(N_JUNK):
            nc.tensor.matmul(scr[:], junk[:, :128], junk[:],
                             start=True, stop=True, skip_group_check=True)
        # --- PE: the real packed Gram accumulation.
        for i in range(N_TILES):
            nc.tensor.wait_ge(dma_sems[i], 16)
            pairs = TILE_FREE[i] // (2 * S)
            for j in range(pairs):
                c = xbuf[:, TILE_OFF[i] + j * 2 * S:
                         TILE_OFF[i] + (j + 1) * 2 * S]
                mm = nc.tensor.matmul(
                    acc[:], c, c,
                    start=(i == 0 and j == 0),
                    stop=(i == N_TILES - 1 and j == pairs - 1),
                )
                if i == N_TILES - 1 and j == pairs - 1:
                    mm.then_inc(pe_sem, 1)

        # --- DVE: merge the diagonal blocks.
        nc.vector.wait_ge(pe_sem, 1)
        nc.vector.tensor_copy(obuf[:], acc[:S, :S])
        nc.vector.tensor_add(obuf[:], obuf[:],
                             acc[S:, S:]).then_inc(out_sem, 1)

        # --- SP: store the partial Gram (receipt lands on S[48],
        # zeroed by the NRT postamble; nothing on-device waits for it).
        nc.sync.wait_ge(out_sem, 1)
        nc.sync.dma_start(g[:], obuf[:]).then_inc(
            _STORE_SEM, 16, skip_validation=True)

    nc.compile()
    return nc


def get_nc():
    if "nc" not in _CACHE:
        _CACHE["nc"] = _build_nc()
    return _CACHE["nc"]


def _device_partial_grams(flat8, **run_kwargs) -> np.ndarray:
    """Run the SPMD bass kernel; return the 8 partial Grams [8, 64, 64]."""
    global LAST_RESULTS
    nc = get_nc()
    in_maps = [
        {"x": flat8[c * N_SHARD:(c + 1) * N_SHARD]} for c in range(N_CORES)
    ]
    LAST_RESULTS = run_bass_kernel_spmd(
        nc, in_maps, core_ids=list(range(N_CORES)), **run_kwargs
    )
    return np.stack([LAST_RESULTS.results[c]["g"] for c in range(N_CORES)])


def kernel(input: np.ndarray, **run_kwargs) -> np.ndarray:
    flat = np.asarray(input, dtype=np.float32).reshape(N_TOTAL, S)
    flat8 = np.ascontiguousarray(flat.astype(ml_dtypes.float8_e4m3))
    partials = _device_partial_grams(flat8, **run_kwargs)

    gram = partials.astype(np.float64).sum(axis=0)
    sq = np.diag(gram)
    dist = sq[:, None] + sq[None, :] - 2.0 * gram
    idx = np.arange(S)
    lower = idx[:, None] > idx[None, :]
    adjacent = (idx[:, None] - idx[None, :]) == 1
    per_pair = np.where(adjacent, np.maximum(0.0, MARGIN - dist), dist)
    loss = np.where(lower, per_pair, 0.0).sum() / (S * (S - 1) * 1000)
    return np.asarray(loss, dtype=np.float32)


# revision 13
# speedup vs baseline: 1.0661x; 1.0661x over previous
"""Contrastive-loss kernel for trn2 (8 NeuronCores, SPMD), v4.

The reference loss reduces to a Gram matrix G = F.T @ F over the
flattened input F [N=524288, T=64], followed by a tiny [64,64] masked
margin reduction (host).  Changes vs the 69us baseline:

1. Host-side cast fp32 -> fp8 e4m3 (ml_dtypes.float8_e4m3, the TRN
   fp8e4 format, max +-240).  Device HBM traffic drops 4x to 4.19
   MiB/core (fully drains in ~11.5us at ~368 GB/s/side, all 16 SDMA
   engines ~97% busy); end-to-end loss rel-err ~7e-4 (tolerance 2e-2).
   The PE is the bottleneck.
2. PE warm-up: the HAM clock gate keeps the PE at 1.2 GHz until it has
   been busy for a full ~3.4us activity window (trace: 55 cold matmuls
   = 5.9us wasted).  A handful of junk matmuls on an uninitialized
   SBUF scratch (into a scratch PSUM bank nobody reads) fill the
   NRT-preamble -> first-tile-landed dead window so the HAM flip
   happens during warm-up, not mid-stream.
3. Packed matmuls: lhsT = rhs = [A|B] ([128,128] fp8 -> FWL),
   accumulating [[A'A,A'B],[B'A,B'B]] into one [128,128] PSUM tile;
   diagonal blocks summed by DVE (copy+add) at the end.  256 matmuls
   at ~50ns warm cadence.
4. No nc.Block(): instructions are emitted straight into the entry
   block, which removes the per-engine branch (and its ~0.9us ifetch
   stall on gpsimd right before the first DMA) and the walrus
   end-of-block barrier.
5. Minimal tail: the NRT-injected postamble zeroes ALL 256 semaphores
   (5 engines x 51 sems, trace-verified), so the kernel does no sem
   clearing of its own.  The output store's mandatory semaphore update
   (walrus crashes on a DMACopy with an empty update list) lands on
   forged low sem S[48], which nothing waits on and the postamble
   zeroes anyway -- no engine ever waits for the store's HBM write
   receipt.  gpsimd resets the SWDGE queue state after the last input
   DMA completes, off the critical path.

The 8 partial [64,64] Grams are summed on the host, where the masked
margin reduction (negligible work) also runs.
"""

import contextlib

import numpy as np
import ml_dtypes

import bass_rust
import concourse.bacc as bacc
import concourse.mybir as mybir
from concourse.bass_utils import run_bass_kernel_spmd

# See item 5 above: receipt sink for the output store.  S[48] is only
# ever touched by the NRT postamble reset chains (trace-verified).
_STORE_SEM = bass_rust.SemaphoreHandle("nrt_scratch", 48)

MARGIN = 60000.0
S = 64                           # time steps (Gram dim)
N_TOTAL = 2 * 8 * 32 * 32 * 32   # 524288 flattened rows
N_CORES = 8
N_SHARD = N_TOTAL // N_CORES     # 65536 rows per core
P = 128                          # SBUF partitions
# Tile sizes in rows (multiples of 256 so each tile is a whole number
# of packed [128,128] matmuls).  Equal mid-size tiles: a tile's landing
# time has a ~1.2us fixed floor (per-descriptor cost, 8 descs/engine),
# so tiny lead-in tiles land no sooner than an 8192-row tile but leave
# the PE starved at every boundary (v4 trace: 1.8/1.6/1.0us stalls that
# also re-cooled the HAM clock gate).  The junk warm-up (below) covers
# the lead-in instead, and 8192-row tiles keep the per-tile DMA time
# (~1.4us) below the warm PE time per tile (~1.66us) so the stream
# never stalls after tile 0.  Last tiles slightly bigger for margin.
TILE_ROWS = [8192, 8192, 8192, 8192, 8192, 8192, 8192, 4096, 4096]
# (Splitting the first tiles onto the SP HWDGE ring was tried and made
# the worst core ~3us slower -- mixing the HWDGE and SWDGE queues
# degrades the SDMA round-robin -- so everything goes via gpsimd.)
N_SYNC_TILES = 0
assert sum(TILE_ROWS) == N_SHARD and all(r % 256 == 0 for r in TILE_ROWS)
TILE_FREE = [(r // P) * S for r in TILE_ROWS]   # fp8 elems per partition
TILE_OFF = [sum(TILE_FREE[:i]) for i in range(len(TILE_ROWS))]
XBUF_FREE = sum(TILE_FREE)                      # 32768 B/partition (fp8)
N_TILES = len(TILE_ROWS)
# Junk warm-up matmuls: bridge PE-preamble-end -> tile0-landed (~3.4us)
# with N=512 matmuls (427ns each at the cold 1.2 GHz clock); by the
# time real matmuls start the HAM window has flipped to 2.4 GHz.
N_JUNK = 8

_CACHE = {}
LAST_RESULTS = None              # BassKernelResults of the most recent run


def _build_nc():
    nc = bacc.Bacc("TRN2", target_bir_lowering=False, debug=False,
                   num_devices=N_CORES)
    # Drop the const-AP memsets and the all-engine barrier that
    # Bass.__init__ appends to the entry block (~0.5us before the first
    # kernel instruction can issue).  Nothing in this kernel uses the
    # const APs, and all cross-engine ordering is explicit via sems.
    entry = nc.main_func.blocks[0]
    first_memset = next(i for i, inst in enumerate(entry.instructions)
                        if isinstance(inst, mybir.InstMemset))
    del entry.instructions[first_memset:]

    x = nc.dram_tensor("x", [N_SHARD, S], mybir.dt.float8e4,
                       kind="ExternalInput")
    g = nc.dram_tensor("g", [S, S], mybir.dt.float32, kind="ExternalOutput")

    def tile_src(i):
        a = sum(TILE_ROWS[:i])
        return x[a:a + TILE_ROWS[i]].rearrange(
            "(p r) c -> p (r c)", p=P, r=TILE_ROWS[i] // P)

    with (
        nc.sbuf_tensor("xbuf", [P, XBUF_FREE], mybir.dt.float8e4) as xbuf,
        nc.sbuf_tensor("junk", [P, 512], mybir.dt.float8e4) as junk,
        nc.psum_tensor("acc", [2 * S, 2 * S], mybir.dt.float32) as acc,
        nc.psum_tensor("scr", [P, 512], mybir.dt.float32) as scr,
        nc.sbuf_tensor("obuf", [S, S], mybir.dt.float32) as obuf,
        nc.semaphore("pe_sem") as pe_sem,
        nc.semaphore("out_sem") as out_sem,
        contextlib.ExitStack() as stack,
    ):
        dma_sems = [stack.enter_context(nc.semaphore(f"dma_sem{k}"))
                    for k in range(N_TILES)]
        dma_lo = min(s.num for s in dma_sems)
        dma_hi = max(s.num for s in dma_sems)
        assert dma_hi - dma_lo == N_TILES - 1

        # --- SP: the first tiles via HWDGE (earliest possible start).
        for i in range(N_SYNC_TILES):
            nc.sync.dma_start(
                xbuf[:, TILE_OFF[i]:TILE_OFF[i] + TILE_FREE[i]],
                tile_src(i),
            ).then_inc(dma_sems[i], 16)
        # --- gpsimd: the bulk of the input via SWDGE, then reset the
        # SWDGE queue state.
        for i in range(N_SYNC_TILES, N_TILES):
            nc.gpsimd.dma_start(
                xbuf[:, TILE_OFF[i]:TILE_OFF[i] + TILE_FREE[i]],
                tile_src(i),
            ).then_inc(dma_sems[i], 16)
        # dma_sems[-1] == 16 implies every engine drained its FIFO
        # through the last tile, i.e. all SWDGE input DMAs completed (it
        # does NOT touch sem values, so the PE's pending per-tile waits
        # are unaffected).
        nc.gpsimd.wait_ge(dma_sems[-1], 16)
        nc.gpsimd.dma_reset(range(dma_lo, dma_hi + 1))

        # --- PE: junk warm-up (uninitialized operands, scratch PSUM,
        # nobody reads the result -- only the HAM activity matters).
        for j in range(N_JUNK):
            nc.tensor.matmul(scr[:], junk[:, :128], junk[:],
                             start=True, stop=True, skip_group_check=True)
        # --- PE: the real packed Gram accumulation.
        for i in range(N_TILES):
            nc.tensor.wait_ge(dma_sems[i], 16)
            pairs = TILE_FREE[i] // (2 * S)
            for j in range(pairs):
                c = xbuf[:, TILE_OFF[i] + j * 2 * S:
                         TILE_OFF[i] + (j + 1) * 2 * S]
                mm = nc.tensor.matmul(
                    acc[:], c, c,
                    start=(i == 0 and j == 0),
                    stop=(i == N_TILES - 1 and j == pairs - 1),
                )
                if i == N_TILES - 1 and j == pairs - 1:
                    mm.then_inc(pe_sem, 1)

        # --- DVE: merge the diagonal blocks.
        nc.vector.wait_ge(pe_sem, 1)
        nc.vector.tensor_copy(obuf[:], acc[:S, :S])
        nc.vector.tensor_add(obuf[:], obuf[:],
                             acc[S:, S:]).then_inc(out_sem, 1)

        # --- SP: store the partial Gram (receipt lands on S[48],
        # zeroed by the NRT postamble; nothing on-device waits for it).
        nc.sync.wait_ge(out_sem, 1)
        nc.sync.dma_start(g[:], obuf[:]).then_inc(
            _STORE_SEM, 16, skip_validation=True)

    nc.compile()
    return nc


def get_nc():
    if "nc" not in _CACHE:
        _CACHE["nc"] = _build_nc()
    return _CACHE["nc"]


def _device_partial_grams(flat8, **run_kwargs) -> np.ndarray:
    """Run the SPMD bass kernel; return the 8 partial Grams [8, 64, 64]."""
    global LAST_RESULTS
    nc = get_nc()
    in_maps = [
        {"x": flat8[c * N_SHARD:(c + 1) * N_SHARD]} for c in range(N_CORES)
    ]
    LAST_RESULTS = run_bass_kernel_spmd(
        nc, in_maps, core_ids=list(range(N_CORES)), **run_kwargs
    )
    return np.stack([LAST_RESULTS.results[c]["g"] for c in range(N_CORES)])


def kernel(input: np.ndarray, **run_kwargs) -> np.ndarray:
    flat = np.asarray(input, dtype=np.float32).reshape(N_TOTAL, S)
    flat8 = np.ascontiguousarray(flat.astype(ml_dtypes.float8_e4m3))
    partials = _device_partial_grams(flat8, **run_kwargs)

    gram = partials.astype(np.float64).sum(axis=0)
    sq = np.diag(gram)
    dist = sq[:, None] + sq[None, :] - 2.0 * gram
    idx = np.arange(S)
    lower = idx[:, None] > idx[None, :]
    adjacent = (idx[:, None] - idx[None, :]) == 1
    per_pair = np.where(adjacent, np.maximum(0.0, MARGIN - dist), dist)
    loss = np.where(lower, per_pair, 0.0).sum() / (S * (S - 1) * 1000)
    return np.asarray(loss, dtype=np.float32)


# revision 15
# speedup vs baseline: 1.0719x; 1.0055x over previous
"""Contrastive-loss kernel for trn2 (8 NeuronCores, SPMD), v4.

The reference loss reduces to a Gram matrix G = F.T @ F over the
flattened input F [N=524288, T=64], followed by a tiny [64,64] masked
margin reduction (host).  Changes vs the 69us baseline:

1. Host-side cast fp32 -> fp8 e4m3 (ml_dtypes.float8_e4m3, the TRN
   fp8e4 format, max +-240).  Device HBM traffic drops 4x to 4.19
   MiB/core (fully drains in ~11.5us at ~368 GB/s/side, all 16 SDMA
   engines ~97% busy); end-to-end loss rel-err ~7e-4 (tolerance 2e-2).
   The PE is the bottleneck.
2. PE warm-up: the HAM clock gate keeps the PE at 1.2 GHz until it has
   been busy for a full ~3.4us activity window (trace: 55 cold matmuls
   = 5.9us wasted).  A handful of junk matmuls on an uninitialized
   SBUF scratch (into a scratch PSUM bank nobody reads) fill the
   NRT-preamble -> first-tile-landed dead window so the HAM flip
   happens during warm-up, not mid-stream.
3. Packed matmuls: lhsT = rhs = [A|B] ([128,128] fp8 -> FWL),
   accumulating [[A'A,A'B],[B'A,B'B]] into one [128,128] PSUM tile;
   diagonal blocks summed by DVE (copy+add) at the end.  256 matmuls
   at ~50ns warm cadence.
4. No nc.Block(): instructions are emitted straight into the entry
   block, which removes the per-engine branch (and its ~0.9us ifetch
   stall on gpsimd right before the first DMA) and the walrus
   end-of-block barrier.
5. Minimal tail: the NRT-injected postamble zeroes ALL 256 semaphores
   (5 engines x 51 sems, trace-verified), so the kernel does no sem
   clearing of its own.  The output store's mandatory semaphore update
   (walrus crashes on a DMACopy with an empty update list) lands on
   forged low sem S[48], which nothing waits on and the postamble
   zeroes anyway -- no engine ever waits for the store's HBM write
   receipt.  gpsimd resets the SWDGE queue state after the last input
   DMA completes, off the critical path.

The 8 partial [64,64] Grams are summed on the host, where the masked
margin reduction (negligible work) also runs.
"""

import contextlib

import numpy as np
import ml_dtypes

import bass_rust
import concourse.bacc as bacc
import concourse.mybir as mybir
from concourse.bass_utils import run_bass_kernel_spmd

# See item 5 above: receipt sink for the output store.  S[48] is only
# ever touched by the NRT postamble reset chains (trace-verified).
_STORE_SEM = bass_rust.SemaphoreHandle("nrt_scratch", 48)

MARGIN = 60000.0
S = 64                           # time steps (Gram dim)
N_TOTAL = 2 * 8 * 32 * 32 * 32   # 524288 flattened rows
N_CORES = 8
N_SHARD = N_TOTAL // N_CORES     # 65536 rows per core
P = 128                          # SBUF partitions
# Tile sizes in rows (multiples of 256 so each tile is a whole number
# of packed [128,128] matmuls).  The SWDGE ramp delivers the first
# ~1 MiB slowly (~175 GB/s vs ~275-360 steady), so a short staircase of
# small tiles lets the PE start ~1.5us earlier and ride the ramp; the
# 8192-row steady tiles keep per-tile DMA time (~1.4us) below the warm
# PE time per tile (~1.8us) so the stream does not stall mid-flight.
# (The junk warm-up below keeps the HAM clock gate busy through any
# residual early-boundary wait -- tiny tiles with NO warm-up stalled
# 1-2us per boundary and re-cooled the PE clock in an earlier rev.)
TILE_ROWS = [2048, 2048, 4096, 8192, 8192, 8192, 8192, 8192, 8192, 8192]
# (Splitting the first tiles onto the SP HWDGE ring was tried and made
# the worst core ~3us slower -- mixing the HWDGE and SWDGE queues
# degrades the SDMA round-robin -- so everything goes via gpsimd.)
N_SYNC_TILES = 0
assert sum(TILE_ROWS) == N_SHARD and all(r % 256 == 0 for r in TILE_ROWS)
TILE_FREE = [(r // P) * S for r in TILE_ROWS]   # fp8 elems per partition
TILE_OFF = [sum(TILE_FREE[:i]) for i in range(len(TILE_ROWS))]
XBUF_FREE = sum(TILE_FREE)                      # 32768 B/partition (fp8)
N_TILES = len(TILE_ROWS)
# Junk warm-up matmuls: bridge PE-preamble-end -> tile0-landed (~3.4us)
# with N=512 matmuls (427ns each at the cold 1.2 GHz clock); by the
# time real matmuls start the HAM window has flipped to 2.4 GHz.
N_JUNK = 8

_CACHE = {}
LAST_RESULTS = None              # BassKernelResults of the most recent run


def _build_nc():
    nc = bacc.Bacc("TRN2", target_bir_lowering=False, debug=False,
                   num_devices=N_CORES)
    # Drop the const-AP memsets and the all-engine barrier that
    # Bass.__init__ appends to the entry block (~0.5us before the first
    # kernel instruction can issue).  Nothing in this kernel uses the
    # const APs, and all cross-engine ordering is explicit via sems.
    entry = nc.main_func.blocks[0]
    first_memset = next(i for i, inst in enumerate(entry.instructions)
                        if isinstance(inst, mybir.InstMemset))
    del entry.instructions[first_memset:]

    x = nc.dram_tensor("x", [N_SHARD, S], mybir.dt.float8e4,
                       kind="ExternalInput")
    g = nc.dram_tensor("g", [S, S], mybir.dt.float32, kind="ExternalOutput")

    def tile_src(i):
        a = sum(TILE_ROWS[:i])
        return x[a:a + TILE_ROWS[i]].rearrange(
            "(p r) c -> p (r c)", p=P, r=TILE_ROWS[i] // P)

    with (
        nc.sbuf_tensor("xbuf", [P, XBUF_FREE], mybir.dt.float8e4) as xbuf,
        nc.sbuf_tensor("junk", [P, 512], mybir.dt.float8e4) as junk,
        nc.psum_tensor("acc", [2 * S, 2 * S], mybir.dt.float32) as acc,
        nc.psum_tensor("scr", [P, 512], mybir.dt.float32) as scr,
        nc.sbuf_tensor("obuf", [S, S], mybir.dt.float32) as obuf,
        nc.semaphore("pe_sem") as pe_sem,
        nc.semaphore("out_sem") as out_sem,
        contextlib.ExitStack() as stack,
    ):
        dma_sems = [stack.enter_context(nc.semaphore(f"dma_sem{k}"))
                    for k in range(N_TILES)]
        dma_lo = min(s.num for s in dma_sems)
        dma_hi = max(s.num for s in dma_sems)
        assert dma_hi - dma_lo == N_TILES - 1

        # --- SP: the first tiles via HWDGE (earliest possible start).
        for i in range(N_SYNC_TILES):
            nc.sync.dma_start(
                xbuf[:, TILE_OFF[i]:TILE_OFF[i] + TILE_FREE[i]],
                tile_src(i),
            ).then_inc(dma_sems[i], 16)
        # --- gpsimd: the bulk of the input via SWDGE, then reset the
        # SWDGE queue state.
        for i in range(N_SYNC_TILES, N_TILES):
            nc.gpsimd.dma_start(
                xbuf[:, TILE_OFF[i]:TILE_OFF[i] + TILE_FREE[i]],
                tile_src(i),
            ).then_inc(dma_sems[i], 16)
        # dma_sems[-1] == 16 implies every engine drained its FIFO
        # through the last tile, i.e. all SWDGE input DMAs completed (it
        # does NOT touch sem values, so the PE's pending per-tile waits
        # are unaffected).
        nc.gpsimd.wait_ge(dma_sems[-1], 16)
        nc.gpsimd.dma_reset(range(dma_lo, dma_hi + 1))

        # --- PE: junk warm-up (uninitialized operands, scratch PSUM,
        # nobody reads the result -- only the HAM activity matters).
        for j in range(N_JUNK):
            nc.tensor.matmul(scr[:], junk[:, :128], junk[:],
                             start=True, stop=True, skip_group_check=True)
        # --- PE: the real packed Gram accumulation.
        for i in range(N_TILES):
            nc.tensor.wait_ge(dma_sems[i], 16)
            pairs = TILE_FREE[i] // (2 * S)
            for j in range(pairs):
                c = xbuf[:, TILE_OFF[i] + j * 2 * S:
                         TILE_OFF[i] + (j + 1) * 2 * S]
                mm = nc.tensor.matmul(
                    acc[:], c, c,
                    start=(i == 0 and j == 0),
                    stop=(i == N_TILES - 1 and j == pairs - 1),
                )
                if i == N_TILES - 1 and j == pairs - 1:
                    mm.then_inc(pe_sem, 1)

        # --- DVE: merge the diagonal blocks.
        nc.vector.wait_ge(pe_sem, 1)
        nc.vector.tensor_copy(obuf[:], acc[:S, :S])
        nc.vector.tensor_add(obuf[:], obuf[:],
                             acc[S:, S:]).then_inc(out_sem, 1)

        # --- SP: store the partial Gram (receipt lands on S[48],
        # zeroed by the NRT postamble; nothing on-device waits for it).
        nc.sync.wait_ge(out_sem, 1)
        nc.sync.dma_start(g[:], obuf[:]).then_inc(
            _STORE_SEM, 16, skip_validation=True)

    nc.compile()
    return nc


def get_nc():
    if "nc" not in _CACHE:
        _CACHE["nc"] = _build_nc()
    return _CACHE["nc"]


def _device_partial_grams(flat8, **run_kwargs) -> np.ndarray:
    """Run the SPMD bass kernel; return the 8 partial Grams [8, 64, 64]."""
    global LAST_RESULTS
    nc = get_nc()
    in_maps = [
        {"x": flat8[c * N_SHARD:(c + 1) * N_SHARD]} for c in range(N_CORES)
    ]
    LAST_RESULTS = run_bass_kernel_spmd(
        nc, in_maps, core_ids=list(range(N_CORES)), **run_kwargs
    )
    return np.stack([LAST_RESULTS.results[c]["g"] for c in range(N_CORES)])


def kernel(input: np.ndarray, **run_kwargs) -> np.ndarray:
    flat = np.asarray(input, dtype=np.float32).reshape(N_TOTAL, S)
    flat8 = np.ascontiguousarray(flat.astype(ml_dtypes.float8_e4m3))
    partials = _device_partial_grams(flat8, **run_kwargs)

    gram = partials.astype(np.float64).sum(axis=0)
    sq = np.diag(gram)
    dist = sq[:, None] + sq[None, :] - 2.0 * gram
    idx = np.arange(S)
    lower = idx[:, None] > idx[None, :]
    adjacent = (idx[:, None] - idx[None, :]) == 1
    per_pair = np.where(adjacent, np.maximum(0.0, MARGIN - dist), dist)
    loss = np.where(lower, per_pair, 0.0).sum() / (S * (S - 1) * 1000)
    return np.asarray(loss, dtype=np.float32)


# revision 18
# speedup vs baseline: 1.0739x; 1.0019x over previous
"""Contrastive-loss kernel for trn2 (8 NeuronCores, SPMD).

The reference loss reduces to a Gram matrix G = F.T @ F over the
flattened input F [N=524288, T=64], followed by a tiny [64,64] masked
margin reduction (host).  ~69us (baseline) -> ~34.5us.  Changes vs the
baseline:

1. Host-side cast fp32 -> fp8 e4m3 (ml_dtypes.float8_e4m3, the TRN
   fp8e4 format, max +-240).  Device HBM traffic drops 4x to 4.19
   MiB/core (drains in ~11.5-15us; 8 cores together sit at the chip
   HBM ceiling, so per-core rate varies 240-370 GB/s run to run);
   end-to-end loss rel-err ~7e-4 (tolerance 2e-2).  The PE matmul
   stream (~14.3us warm) and the DMA are roughly balanced.
2. PE warm-up: the HAM clock gate keeps the PE at 1.2 GHz until it has
   been busy for a full ~3.4us activity window (trace: 55 cold matmuls
   = 5.9us wasted).  A handful of junk matmuls on an uninitialized
   SBUF scratch (into a scratch PSUM bank nobody reads) fill the
   NRT-preamble -> first-tile-landed dead window so the HAM flip
   happens during warm-up, not mid-stream.
3. Packed matmuls: lhsT = rhs = [A|B] ([128,128] fp8 -> FWL),
   accumulating [[A'A,A'B],[B'A,B'B]] into one [128,128] PSUM tile;
   diagonal blocks summed by DVE (copy+add) at the end.  256 matmuls
   at ~50ns warm cadence.
4. No nc.Block(): instructions are emitted straight into the entry
   block, which removes the per-engine branch (and its ~0.9us ifetch
   stall on gpsimd right before the first DMA) and the walrus
   end-of-block barrier.
5. Minimal tail: the NRT-injected postamble zeroes ALL 256 semaphores
   (5 engines x 51 sems, trace-verified), so the kernel does no sem
   clearing of its own.  The output store's mandatory semaphore update
   (walrus crashes on a DMACopy with an empty update list) lands on
   forged low sem S[48], which nothing waits on and the postamble
   zeroes anyway -- no engine ever waits for the store's HBM write
   receipt.  gpsimd resets the SWDGE queue state after the last input
   DMA completes, off the critical path.

The 8 partial [64,64] Grams are summed on the host, where the masked
margin reduction (negligible work) also runs.
"""

import contextlib

import numpy as np
import ml_dtypes

import bass_rust
import concourse.bacc as bacc
import concourse.mybir as mybir
from concourse.bass_utils import run_bass_kernel_spmd

# See item 5 above: receipt sink for the output store.  S[48] is only
# ever touched by the NRT postamble reset chains (trace-verified).
_STORE_SEM = bass_rust.SemaphoreHandle("nrt_scratch", 48)

MARGIN = 60000.0
S = 64                           # time steps (Gram dim)
N_TOTAL = 2 * 8 * 32 * 32 * 32   # 524288 flattened rows
N_CORES = 8
N_SHARD = N_TOTAL // N_CORES     # 65536 rows per core
P = 128                          # SBUF partitions
# Tile sizes in rows (multiples of 256 so each tile is a whole number
# of packed [128,128] matmuls).  The SWDGE ramp delivers the first
# ~1 MiB slowly (~175 GB/s vs ~275-360 steady), so a short staircase of
# small tiles lets the PE start ~1.5us earlier and ride the ramp; the
# 8192-row steady tiles keep per-tile DMA time (~1.4us) below the warm
# PE time per tile (~1.8us) so the stream does not stall mid-flight.
# (The junk warm-up below keeps the HAM clock gate busy through any
# residual early-boundary wait -- tiny tiles with NO warm-up stalled
# 1-2us per boundary and re-cooled the PE clock in an earlier rev.)
TILE_ROWS = [8192, 8192, 8192, 8192, 8192, 12288, 12288]
# (Variants tried and rejected on hardware, all within ~1us of noise or
# worse on the worst core: a 2048/2048/4096 lead-in staircase, 4096
# tail tiles, and issuing the first tiles from the SP HWDGE ring --
# that last one cost the worst core ~3us, as mixing HWDGE and SWDGE
# queues degrades the SDMA round-robin.)
N_SYNC_TILES = 0
assert sum(TILE_ROWS) == N_SHARD and all(r % 256 == 0 for r in TILE_ROWS)
TILE_FREE = [(r // P) * S for r in TILE_ROWS]   # fp8 elems per partition
TILE_OFF = [sum(TILE_FREE[:i]) for i in range(len(TILE_ROWS))]
XBUF_FREE = sum(TILE_FREE)                      # 32768 B/partition (fp8)
N_TILES = len(TILE_ROWS)
# Junk warm-up matmuls: bridge PE-preamble-end -> tile0-landed (~3.4us)
# with N=512 matmuls (427ns each at the cold 1.2 GHz clock); by the
# time real matmuls start the HAM window has flipped to 2.4 GHz.
N_JUNK = 8

_CACHE = {}
LAST_RESULTS = None              # BassKernelResults of the most recent run


def _build_nc():
    nc = bacc.Bacc("TRN2", target_bir_lowering=False, debug=False,
                   num_devices=N_CORES)
    # Drop the const-AP memsets and the all-engine barrier that
    # Bass.__init__ appends to the entry block (~0.5us before the first
    # kernel instruction can issue).  Nothing in this kernel uses the
    # const APs, and all cross-engine ordering is explicit via sems.
    entry = nc.main_func.blocks[0]
    first_memset = next(i for i, inst in enumerate(entry.instructions)
                        if isinstance(inst, mybir.InstMemset))
    del entry.instructions[first_memset:]

    x = nc.dram_tensor("x", [N_SHARD, S], mybir.dt.float8e4,
                       kind="ExternalInput")
    g = nc.dram_tensor("g", [S, S], mybir.dt.float32, kind="ExternalOutput")

    def tile_src(i):
        a = sum(TILE_ROWS[:i])
        return x[a:a + TILE_ROWS[i]].rearrange(
            "(p r) c -> p (r c)", p=P, r=TILE_ROWS[i] // P)

    with (
        nc.sbuf_tensor("xbuf", [P, XBUF_FREE], mybir.dt.float8e4) as xbuf,
        nc.sbuf_tensor("junk", [P, 512], mybir.dt.float8e4) as junk,
        nc.psum_tensor("acc", [2 * S, 2 * S], mybir.dt.float32) as acc,
        nc.psum_tensor("scr", [P, 512], mybir.dt.float32) as scr,
        nc.sbuf_tensor("obuf", [S, S], mybir.dt.float32) as obuf,
        nc.semaphore("pe_sem") as pe_sem,
        nc.semaphore("out_sem") as out_sem,
        contextlib.ExitStack() as stack,
    ):
        dma_sems = [stack.enter_context(nc.semaphore(f"dma_sem{k}"))
                    for k in range(N_TILES)]
        dma_lo = min(s.num for s in dma_sems)
        dma_hi = max(s.num for s in dma_sems)
        assert dma_hi - dma_lo == N_TILES - 1

        # --- SP: the first tiles via HWDGE (earliest possible start).
        for i in range(N_SYNC_TILES):
            nc.sync.dma_start(
                xbuf[:, TILE_OFF[i]:TILE_OFF[i] + TILE_FREE[i]],
                tile_src(i),
            ).then_inc(dma_sems[i], 16)
        # --- gpsimd: the bulk of the input via SWDGE, then reset the
        # SWDGE queue state.
        for i in range(N_SYNC_TILES, N_TILES):
            nc.gpsimd.dma_start(
                xbuf[:, TILE_OFF[i]:TILE_OFF[i] + TILE_FREE[i]],
                tile_src(i),
            ).then_inc(dma_sems[i], 16)
        # dma_sems[-1] == 16 implies every engine drained its FIFO
        # through the last tile, i.e. all SWDGE input DMAs completed (it
        # does NOT touch sem values, so the PE's pending per-tile waits
        # are unaffected).
        nc.gpsimd.wait_ge(dma_sems[-1], 16)
        nc.gpsimd.dma_reset(range(dma_lo, dma_hi + 1))

        # --- PE: junk warm-up (uninitialized operands, scratch PSUM,
        # nobody reads the result -- only the HAM activity matters).
        for j in range(N_JUNK):
            nc.tensor.matmul(scr[:], junk[:, :128], junk[:],
                             start=True, stop=True, skip_group_check=True)
        # --- PE: the real packed Gram accumulation.
        for i in range(N_TILES):
            nc.tensor.wait_ge(dma_sems[i], 16)
            pairs = TILE_FREE[i] // (2 * S)
            for j in range(pairs):
                c = xbuf[:, TILE_OFF[i] + j * 2 * S:
                         TILE_OFF[i] + (j + 1) * 2 * S]
                mm = nc.tensor.matmul(
                    acc[:], c, c,
                    start=(i == 0 and j == 0),
                    stop=(i == N_TILES - 1 and j == pairs - 1),
                )
                if i == N_TILES - 1 and j == pairs - 1:
                    mm.then_inc(pe_sem, 1)

        # --- DVE: merge the diagonal blocks.
        nc.vector.wait_ge(pe_sem, 1)
        nc.vector.tensor_copy(obuf[:], acc[:S, :S])
        nc.vector.tensor_add(obuf[:], obuf[:],
                             acc[S:, S:]).then_inc(out_sem, 1)

        # --- SP: store the partial Gram (receipt lands on S[48],
        # zeroed by the NRT postamble; nothing on-device waits for it).
        nc.sync.wait_ge(out_sem, 1)
        nc.sync.dma_start(g[:], obuf[:]).then_inc(
            _STORE_SEM, 16, skip_validation=True)

    nc.compile()
    return nc


def get_nc():
    if "nc" not in _CACHE:
        _CACHE["nc"] = _build_nc()
    return _CACHE["nc"]


def _device_partial_grams(flat8, **run_kwargs) -> np.ndarray:
    """Run the SPMD bass kernel; return the 8 partial Grams [8, 64, 64]."""
    global LAST_RESULTS
    nc = get_nc()
    in_maps = [
        {"x": flat8[c * N_SHARD:(c + 1) * N_SHARD]} for c in range(N_CORES)
    ]
    LAST_RESULTS = run_bass_kernel_spmd(
        nc, in_maps, core_ids=list(range(N_CORES)), **run_kwargs
    )
    return np.stack([LAST_RESULTS.results[c]["g"] for c in range(N_CORES)])


def kernel(input: np.ndarray, **run_kwargs) -> np.ndarray:
    flat = np.asarray(input, dtype=np.float32).reshape(N_TOTAL, S)
    flat8 = np.ascontiguousarray(flat.astype(ml_dtypes.float8_e4m3))
    partials = _device_partial_grams(flat8, **run_kwargs)

    gram = partials.astype(np.float64).sum(axis=0)
    sq = np.diag(gram)
    dist = sq[:, None] + sq[None, :] - 2.0 * gram
    idx = np.arange(S)
    lower = idx[:, None] > idx[None, :]
    adjacent = (idx[:, None] - idx[None, :]) == 1
    per_pair = np.where(adjacent, np.maximum(0.0, MARGIN - dist), dist)
    loss = np.where(lower, per_pair, 0.0).sum() / (S * (S - 1) * 1000)
    return np.asarray(loss, dtype=np.float32)
